# revision 33
# baseline (speedup 1.0000x reference)
"""Trainium2 Bass kernel for nn_AcceptHead: out = fc2(gelu(fc1(LN(x)))).

Self-contained: hardcodes shapes (B=4, L=4096, H=4096, F=1024) and the
data-parallel sharding (tokens split 8 ways, head params replicated).

"LN-fold" architecture: the device PE stream is *only* fc1 matmuls.
LayerNorm is folded into the matmul epilogue:

    LN(x)@W = r_t * (x@W) - r_t * mu_t * colsum(W)     (gamma folded into W)

  - x arrives already TRANSPOSED from the host (xts: [128, tile, k, tok]
    fp16, t-tile-blocked) -- no on-device transpose, no normalize pass.
  - fc1: lhsT = xT block [128h x 128t], rhs = w1ts [128h x 512f], PSUM
    accumulation over 32 k-tiles (fp16 -- the PE floor, ~218us @2.4GHz).
  - mean correction: one K=1 matmul per (t-tile, f-half) adds
    (-mu_t) * colsum_f into the same PSUM group (lhsT = row of -mu at
    partition 0 made by a tiny PE transpose; rhs = colsum row).
  - rstd r_t is applied as the per-partition `scale` AP of the Gelu
    activation: g = Gelu(r * psum). Newton rsqrt on DVE (bit-trick seed).
  - stats (sum via DVE reduce / sumsq via ACT Square accum_out) come from
    a second wire copy of x in [tok, H] fp8 layout; stats are emitted a
    full chunk ahead so the in-order DVE queue never blocks the PE.
  - fc2 as DVE dot (g * w2_bcast, reduce); out written [tile, 128] via a
    final PE transpose so the out DMA is 16 contiguous rows.

Schedule notes (measured on HW via ntff traces):
  - DMA bandwidth is SHARED (~350 GB/s/core across all queues), so the
    fill is bandwidth-bound: stage 1 = w1ts f-half 0 (SP lo-k / SWDGE
    hi-k) + first xts tiles (ACT ring), stage 2 = f-half 1, stage 3 = xs
    + later chunks. k is consumed lo/hi-interleaved (k0,k16,k1,...) so a
    slow queue half doesn't stall the chase.
  - chunk layout [3,2,2,2,2,2,2,1] tiles: a 3-tile fh0-major first chunk
    amortizes the stage-1 wait across 20us of matmuls (PSUM allows 3
    open accumulators: 6 banks + 1 transpose + 1 out = 8).
  - Any PE idle gap triggers a HAM re-throttle (K=8->4 rows, ~7us at
    half speed), so gap avoidance pays double.
  - per-(tile,fh) epilogues interleave right behind each tile's k-loops.

fp8 matmul was investigated and rejected: DoubleRow measures 2x fp16 per
unit contraction on this HW (3826ns vs 7386ns per K=4096,N=512 group;
the cost model's 0.5 cyc/row = 4x is wrong here), and at 2x every
precision-passing fp8 scheme costs the same as fp16 (1-pass e4m3 fails
the 2e-2 gate at 4.2e-2 measured in simulation).
"""

import os
import sys

for _p in ("/opt/trn_rl_repo", "/root/.axon_site/_ro/trn_rl_repo"):
    if os.path.isdir(_p) and _p not in sys.path:
        sys.path.append(_p)

import numpy as np

import concourse.bacc as bacc
import concourse.mybir as mybir
import concourse.tile as tile
from concourse.bass_utils import run_bass_kernel_spmd

N_CORES = 8
B, L, H = 4, 4096, 4096
F = H // 4
F2 = F // 2                   # 512, f-half width
T_TOT = B * L                 # 16384 tokens
T_CORE = T_TOT // N_CORES     # 2048 tokens per core
P = 128
KT = H // P                   # 32 contraction tiles
N_TTILES = T_CORE // P        # 16
EPS = 1e-5
RSQRT_MAGIC = 0x5F3759DF

# tiles per pipeline chunk: 3-tile fh-major fill chunk, then 2-tile
# steady chunks, 1-tile last chunk (shortens the drain chain)
CHUNK_TILES = [[0, 1, 2]] + [[t, t + 1] for t in range(3, 15, 2)] + [[15]]
MAXTT = 3

F16 = mybir.dt.float16
F32 = mybir.dt.float32
F8 = mybir.dt.float8e4
I32 = mybir.dt.int32
AF = mybir.ActivationFunctionType
ALU = mybir.AluOpType


def build_program(has_bias1: bool, bias2_val: float):
    nc = bacc.Bacc(
        "TRN2",
        target_bir_lowering=False,
        debug=False,
        enable_asserts=False,
        num_devices=N_CORES,
    )
    # x, transposed + t-tile-blocked on host: xts[p, n, k, t] = x[n*128+t, k*128+p]
    xts_d = nc.dram_tensor(
        "xts", [P, N_TTILES, KT, P], F16, kind="ExternalInput"
    ).ap()
    # x, natural [tok, H] layout (stats only; fp8 halves its wire cost and
    # the ~2.6% quantization only perturbs mu/r by ~4e-4 relative)
    xs_d = nc.dram_tensor("xs", [T_CORE, H], F8, kind="ExternalInput").ap()
    # w1 (gamma-folded, transposed): w1ts[p, fh, k, j] = w1g[k*128+p, fh*512+j]
    w1ts_d = nc.dram_tensor(
        "w1ts", [P, 2, KT, F2], F16, kind="ExternalInput"
    ).ap()
    cs_d = nc.dram_tensor("cs", [1, F], F16, kind="ExternalInput").ap()
    w2b_d = nc.dram_tensor("w2b", [P, F], F16, kind="ExternalInput").ap()
    if has_bias1:
        b1r_d = nc.dram_tensor("b1r", [1, F], F16, kind="ExternalInput").ap()
    # out as [t-tile, partition]: token t = n*128+p lives at out[n, p], so
    # the flattened DRAM tensor IS token order. A direct (n p) -> p n
    # scatter DMA would cost ~10us in 4-byte descriptors.
    out_d = nc.dram_tensor(
        "out", [N_TTILES, P], F32, kind="ExternalOutput"
    ).ap()

    with tile.TileContext(nc) as tc:
        with (
            tc.tile_pool(name="singles", bufs=1) as singles,
            tc.tile_pool(name="xtpool", bufs=8) as xtpool,
            tc.tile_pool(name="xspool", bufs=8) as xspool,
            tc.tile_pool(name="sqscr", bufs=1) as sqscr_pool,
            tc.tile_pool(name="gpool", bufs=2) as gpool,
            tc.tile_pool(name="fc2scr", bufs=1) as fc2scr_pool,
            tc.tile_pool(name="stats", bufs=4) as stats,
            tc.tile_pool(name="nrow", bufs=2) as nrow_pool,
            tc.tile_pool(name="psum", bufs=3, space="PSUM") as psum_pool,
            tc.tile_pool(name="tpsum", bufs=1, space="PSUM") as tpsum_pool,
            tc.tile_pool(name="opsum", bufs=1, space="PSUM") as opsum_pool,
        ):
            w1ts_sb = singles.tile([P, 2, KT, F2], F16)

            def w1_dma(eng, fh, ka, kb):
                eng.dma_start(
                    out=w1ts_sb[:, fh, ka:kb, :], in_=w1ts_d[:, fh, ka:kb, :]
                )

            # stage 1: fh0 split SP (k0..15) / SWDGE (k16..31), first blocks
            # small so the PE's first matmul starts ~1us in
            for ka, kb in [(0, 2), (2, 4)] + [
                (k0, k0 + 4) for k0 in range(4, KT // 2, 4)
            ]:
                w1_dma(nc.sync, 0, ka, kb)
            for ka, kb in [(16, 18), (18, 20)] + [
                (k0, k0 + 4) for k0 in range(20, KT, 4)
            ]:
                w1_dma(nc.gpsimd, 0, ka, kb)
            # stage 2: fh1 on the same queue split
            for k0 in range(0, KT // 2, 4):
                w1_dma(nc.sync, 1, k0, k0 + 4)
            for k0 in range(KT // 2, KT, 4):
                w1_dma(nc.gpsimd, 1, k0, k0 + 4)
            cs_sb = singles.tile([1, F], F16)
            nc.gpsimd.dma_start(out=cs_sb, in_=cs_d)
            w2b_sb = singles.tile([P, F], F16)
            nc.gpsimd.dma_start(out=w2b_sb, in_=w2b_d)
            if has_bias1:
                b1r_sb = singles.tile([1, F], F16)
                nc.gpsimd.dma_start(out=b1r_sb, in_=b1r_d)
            outcols = singles.tile([P, N_TTILES], F32)
            outrow = singles.tile([N_TTILES, P], F32)
            ident = singles.tile([P, P], F16)
            ident32 = singles.tile([P, P], F32)
            from concourse.masks import make_identity
            make_identity(nc, ident[:])
            make_identity(nc, ident32[:])

            # ---- chunk input loads; chunk 0 rides the otherwise-idle ACT
            # ring (in parallel with w1ts on SP/SWDGE), the rest go on SP.
            # xts per-tile, k lo/hi interleaved to match consumption order.
            def load_tile_xt(eng, t, first=False):
                xt = xtpool.tile([P, KT, P], F16, tag="xt")
                if first:
                    blocks = [(0, 2), (16, 18), (2, 4), (18, 20)] + [
                        (k0, k0 + 4)
                        for ka in range(4, KT // 2, 4)
                        for k0 in (ka, KT // 2 + ka)
                    ]
                else:
                    blocks = [(0, 8), (16, 24), (8, 16), (24, 32)]
                for ka, kb in blocks:
                    eng.dma_start(
                        out=xt[:, ka:kb, :], in_=xts_d[:, t, ka:kb, :]
                    )
                return xt

            def load_chunk(ci):
                tiles = CHUNK_TILES[ci]
                eng = nc.scalar if ci == 0 else nc.sync
                xts = [
                    load_tile_xt(eng, t, first=(t == 0)) for t in tiles
                ]
                xss = []
                for t in tiles:
                    xs = xspool.tile([P, H], F8, tag="xs")
                    eng.dma_start(out=xs, in_=xs_d[t * P : (t + 1) * P, :])
                    xss.append(xs)
                return xts, xss

            # ---- stats chain (DVE + ACT), emitted one chunk AHEAD ----
            def emit_stats(xss):
                ntt = len(xss)
                sums = stats.tile([P, MAXTT], F32, tag="sums")
                sq = stats.tile([P, MAXTT], F32, tag="sq")
                for i in range(ntt):
                    nc.vector.reduce_sum(
                        sums[:, i : i + 1], xss[i], axis=mybir.AxisListType.X
                    )
                    sqs = sqscr_pool.tile([P, H], F16, tag="sqs")
                    nc.scalar.activation(
                        out=sqs, in_=xss[i], func=AF.Square,
                        accum_out=sq[:, i : i + 1],
                    )
                sums = sums[:, :ntt]
                sq = sq[:, :ntt]
                mu_t = stats.tile([P, MAXTT], F32, tag="mu")
                mu = mu_t[:, :ntt]
                nc.vector.tensor_scalar_mul(mu, sums, 1.0 / H)
                vv_t = stats.tile([P, MAXTT], F32, tag="vv")
                vv = vv_t[:, :ntt]
                # vv = sq/H - mu^2 + eps
                nc.vector.tensor_tensor(out=vv, in0=mu, in1=mu, op=ALU.mult)
                nc.vector.tensor_scalar(
                    out=vv, in0=vv, scalar1=-1.0, scalar2=EPS,
                    op0=ALU.mult, op1=ALU.add,
                )
                nc.vector.tensor_scalar(
                    out=sq, in0=sq, scalar1=1.0 / H, scalar2=None, op0=ALU.mult
                )
                nc.vector.tensor_tensor(out=vv, in0=vv, in1=sq, op=ALU.add)
                # Newton rsqrt: y0 via bit trick, 2 iterations
                y_t = stats.tile([P, MAXTT], F32, tag="y")
                y = y_t[:, :ntt]
                yi = y[:].bitcast(I32)
                nc.vector.tensor_scalar(
                    out=yi, in0=vv[:].bitcast(I32), scalar1=1, scalar2=None,
                    op0=ALU.arith_shift_right,
                )
                nc.vector.tensor_scalar(
                    out=yi, in0=yi, scalar1=-1, scalar2=RSQRT_MAGIC,
                    op0=ALU.mult, op1=ALU.add,
                )
                h_half_t = stats.tile([P, MAXTT], F32, tag="h_half")
                h_half = h_half_t[:, :ntt]
                nc.vector.tensor_scalar_mul(h_half, vv, 0.5)
                u_t = stats.tile([P, MAXTT], F32, tag="u")
                u = u_t[:, :ntt]
                for _ in range(2):
                    nc.vector.tensor_tensor(out=u, in0=y, in1=y, op=ALU.mult)
                    nc.vector.tensor_tensor(out=u, in0=u, in1=h_half, op=ALU.mult)
                    nc.vector.tensor_scalar(
                        out=u, in0=u, scalar1=-1.0, scalar2=1.5,
                        op0=ALU.mult, op1=ALU.add,
                    )
                    nc.vector.tensor_tensor(out=y, in0=y, in1=u, op=ALU.mult)
                # nmr16 = -mu as fp16 (the corr-matmul lhsT operand)
                nmr16_t = stats.tile([P, MAXTT], F16, tag="nmr16")
                nmr16 = nmr16_t[:, :ntt]
                nc.vector.tensor_scalar_mul(nmr16, mu, -1.0)
                if has_bias1:
                    # invr = sqrt(var+eps) = vv * y; bias row b1_eff enters
                    # PSUM as invr_row.T @ b1r so Gelu's r-scale cancels.
                    invr16_t = stats.tile([P, MAXTT], F16, tag="invr16")
                    invr16 = invr16_t[:, :ntt]
                    nc.vector.tensor_tensor(
                        out=invr16, in0=vv, in1=y, op=ALU.mult
                    )
                else:
                    invr16 = None
                return y_t, nmr16, invr16

            cur = load_chunk(0)
            nxt = load_chunk(1)
            st_cur = emit_stats(cur[1])

            nr = ir = None
            for ci in range(len(CHUNK_TILES)):
                tiles = CHUNK_TILES[ci]
                ntt = len(tiles)
                xt_list, xss = cur
                cur = nxt
                y_t, nmr16, invr16 = st_cur

                # ---- -mu rows to partition 0 via PE transpose; top of
                # chunk body (nmr16 computed a full chunk ago) so the DVE
                # copy retires immediately and never blocks the corr MMs in
                # the in-order DVE queue. Chunk 0's stats land ~20us in, so
                # its rows are emitted after the fh0 groups instead. ----
                def emit_nmr_rows():
                    tps = tpsum_pool.tile([1, MAXTT, P], F16, tag="tps")
                    for i in range(ntt):
                        nc.tensor.transpose(
                            tps[:, i, :], nmr16[:, i : i + 1], ident[:]
                        )
                    nr = nrow_pool.tile([1, MAXTT, P], F16, tag="nr")
                    nc.vector.tensor_copy(out=nr, in_=tps)
                    if has_bias1:
                        tps2 = tpsum_pool.tile([1, MAXTT, P], F16, tag="tps")
                        for i in range(ntt):
                            nc.tensor.transpose(
                                tps2[:, i, :], invr16[:, i : i + 1], ident[:]
                            )
                        ir = nrow_pool.tile([1, MAXTT, P], F16, tag="ir")
                        nc.vector.tensor_copy(out=ir, in_=tps2)
                    else:
                        ir = None
                    return nr, ir

                if ci > 0:
                    nr, ir = emit_nmr_rows()
                if ci + 1 < len(CHUNK_TILES):
                    st_cur = emit_stats(cur[1])

                def emit_group(i, fh, g_ps):
                    fcols = slice(fh * F2, (fh + 1) * F2)
                    # k in lo/hi interleave (k0,k16,k1,k17...) matching the
                    # two DMA queues carrying each f-half
                    for kk in range(KT // 2):
                        for k in (kk, KT // 2 + kk):
                            nc.tensor.matmul(
                                g_ps[:, fcols],
                                lhsT=xt_list[i][:, k, :],
                                rhs=w1ts_sb[:, fh, k, :],
                                start=(kk == 0 and k == 0),
                                stop=False,
                            )

                def emit_epilogue(i, g_ps):
                    for fh in range(2):
                        fcols = slice(fh * F2, (fh + 1) * F2)
                        nc.tensor.matmul(
                            g_ps[:, fcols],
                            lhsT=nr[:, i, :],
                            rhs=cs_sb[:, fcols],
                            start=False,
                            stop=(not has_bias1),
                        )
                        if has_bias1:
                            nc.tensor.matmul(
                                g_ps[:, fcols],
                                lhsT=ir[:, i, :],
                                rhs=b1r_sb[:, fcols],
                                start=False,
                                stop=True,
                            )
                    g_sb = gpool.tile([P, F], F16, tag="g_sb")
                    nc.scalar.activation(
                        out=g_sb, in_=g_ps, func=AF.Gelu,
                        scale=y_t[:, i : i + 1],
                    )
                    fc2s = fc2scr_pool.tile([P, F], F16, tag="fc2s")
                    gi = tiles[i]
                    nc.vector.tensor_tensor(
                        out=fc2s, in0=g_sb, in1=w2b_sb, op=ALU.mult
                    )
                    nc.vector.reduce_sum(
                        outcols[:, gi : gi + 1], fc2s, axis=mybir.AxisListType.X
                    )

                if ci == 0:
                    # fill chunk: all fh0 groups first (stage-1 cargo only),
                    # then -mu rows, then fh1 + epilogue per tile
                    psums = []
                    for i in range(ntt):
                        g_ps = psum_pool.tile([P, F], F32, tag="g_ps")
                        psums.append(g_ps)
                        emit_group(i, 0, g_ps)
                    nr, ir = emit_nmr_rows()
                    for i in range(ntt):
                        emit_group(i, 1, psums[i])
                        emit_epilogue(i, psums[i])
                else:
                    for i in range(ntt):
                        g_ps = psum_pool.tile([P, F], F32, tag="g_ps")
                        emit_group(i, 0, g_ps)
                        emit_group(i, 1, g_ps)
                        emit_epilogue(i, g_ps)

                if ci + 2 < len(CHUNK_TILES):
                    nxt = load_chunk(ci + 2)

            if bias2_val != 0.0:
                nc.vector.tensor_scalar_add(outcols, outcols, bias2_val)
            # transpose [128, 16] -> [16, 128] on the PE so the out DMA is
            # 16 contiguous 512B rows instead of 2048 4-byte descriptors
            otp = opsum_pool.tile([N_TTILES, P], F32, tag="otp")
            nc.tensor.transpose(otp[:], outcols[:], ident32[:])
            nc.vector.tensor_copy(out=outrow, in_=otp)
            nc.sync.dma_start(out=out_d, in_=outrow)

    nc.compile()
    return nc


def _prep_host(hidden_states, ln_gamma, ln_beta, w1, bias1, w2, bias2):
    """Host-side marshalling: dtype casts, layout transposes, exact (fp64)
    folding of the LN affine params into fc1."""
    g64 = np.asarray(ln_gamma, np.float64)
    b64 = np.asarray(ln_beta, np.float64)
    w1_64 = np.asarray(w1, np.float64)
    w1g = np.ascontiguousarray((w1_64 * g64[None, :]).T).astype(np.float16)
    # [4096, 1024] -> [128, 2, 32, 512]: w1ts[p, fh, k, j] = w1g[k*128+p, fh*512+j]
    w1ts = np.ascontiguousarray(
        w1g.reshape(KT, P, 2, F2).transpose(1, 2, 0, 3)
    )
    # colsum of the fp16-quantized folded weights (consistency with device MM)
    cs = np.ascontiguousarray(
        w1g.astype(np.float64).sum(axis=0).reshape(1, F)
    ).astype(np.float16)
    b1_eff = (np.asarray(bias1, np.float64) + w1_64 @ b64).astype(np.float32)
    b1r = b1_eff.reshape(1, F).astype(np.float16)
    w2b = np.broadcast_to(
        np.asarray(w2, np.float64).reshape(1, F).astype(np.float16), (P, F)
    ).copy()
    bias2_val = float(np.asarray(bias2).reshape(-1)[0])
    x2 = np.ascontiguousarray(
        np.asarray(hidden_states, np.float32).reshape(T_TOT, H)
    ).astype(np.float16)
    return x2, w1ts, cs, b1r, w2b, bias2_val


_CACHE = {}


def _get_program(has_bias1, bias2_val):
    key = (has_bias1, bias2_val)
    if key not in _CACHE:
        _CACHE[key] = build_program(has_bias1, bias2_val)
    return _CACHE[key]


def make_in_maps(inputs):
    x2, w1ts, cs, b1r, w2b, bias2_val = _prep_host(**inputs)
    has_bias1 = bool(np.any(np.asarray(b1r) != 0.0))
    in_maps = []
    import ml_dtypes

    for core in range(N_CORES):
        xc = x2[core * T_CORE : (core + 1) * T_CORE]  # [2048, 4096]
        # xts[p, n, k, t] = xc[n*128+t, k*128+p]
        xts = np.ascontiguousarray(
            xc.reshape(N_TTILES, P, KT, P).transpose(3, 0, 2, 1)
        )
        m = {
            "xts": xts,
            "xs": np.ascontiguousarray(xc).astype(ml_dtypes.float8_e4m3),
            "w1ts": w1ts,
            "cs": cs,
            "w2b": w2b,
        }
        if has_bias1:
            m["b1r"] = b1r
        in_maps.append(m)
    return in_maps, has_bias1, bias2_val


def kernel(**inputs) -> np.ndarray:
    in_maps, has_bias1, bias2_val = make_in_maps(inputs)
    nc = _get_program(has_bias1, bias2_val)
    res = run_bass_kernel_spmd(nc, in_maps, core_ids=list(range(N_CORES)))
    out = np.concatenate(
        [np.asarray(res.results[i]["out"]).reshape(-1) for i in range(N_CORES)]
    )
    return out.reshape(B, L).astype(np.float32)
